# revision 6
# baseline (speedup 1.0000x reference)
"""Trainium2 Bass kernel for EquivariantSelfAttention (B=4, N=2048, HID=256, 8 heads).

Sharding: 8 cores = 4 batches x 2 query-halves. Each core computes full
attention for one batch over its 1024 queries (all 2048 keys), plus the
per-token epilogue, fully locally (no collectives).

Device layout is channel-major ("transposed"): all transposes are done on the
host (numpy) during shard prep / output gather, so the device only runs
matmuls + elementwise work on [channels, tokens] tiles.
"""

import sys

if "/opt/trn_rl_repo" not in sys.path:
    sys.path.insert(0, "/opt/trn_rl_repo")

import numpy as np
import ml_dtypes

B, N, HID, NH, HD = 4, 2048, 256, 8, 32
NQ = N // 2          # queries per core
NKT = N // 128       # key tiles
SCALE = float(1.0 / np.sqrt(HD))
BF = ml_dtypes.bfloat16

_CACHE = {}


def _build_nc():
    import concourse.bass as bass
    import concourse.mybir as mybir
    import concourse.tile as tile
    from concourse import bacc
    from concourse.bass import ts

    f32 = mybir.dt.float32
    bf16 = mybir.dt.bfloat16
    AF = mybir.ActivationFunctionType
    OP = mybir.AluOpType
    P = 128

    nc = bacc.Bacc("TRN2", target_bir_lowering=False, debug=False,
                   enable_asserts=False, num_devices=8)

    def din(name, shape, dt):
        return nc.dram_tensor(name, shape, dt, kind="ExternalInput").ap()

    xsT = din("xsT", [2 * P, N], bf16)        # x_scalar^T, all tokens
    xqT = din("xqT", [2 * P, NQ], bf16)       # x_scalar^T, query half
    vq32 = din("vq32", [6 * P, NQ], f32)      # vec^T (query half), f32
    vq16 = din("vq16", [6 * P, NQ], bf16)     # vec^T (query half), bf16
    vkv = din("vkv", [N, 3 * HID], bf16)      # vec token-major, all tokens
    wqT = din("wqT", [HID, HID], bf16)
    wkT = din("wkT", [HID, HID], bf16)
    wvT = din("wvT", [HID, HID], bf16)
    wvecT = din("wvecT", [HID, 2 * HID], bf16)
    woT = din("woT", [HID, 3 * HID], bf16)
    wgT = din("wgT", [2 * HID, HID], bf16)    # alpha-folded
    bq = din("bq", [HID, 1], f32)
    bk = din("bk", [HID, 1], f32)
    bvB = din("bvB", [P, HID], f32)           # bv broadcast over partitions
    bg = din("bg", [HID, 1], f32)
    bo = din("bo", [3 * HID, 1], f32)
    ones = din("ones", [P, P], bf16)
    out = nc.dram_tensor("out", [4 * HID, NQ], f32, kind="ExternalOutput").ap()

    with tile.TileContext(nc) as tc:
        from contextlib import ExitStack
        with ExitStack() as ctx:
            def sb(name, shape, dt):
                return nc.alloc_sbuf_tensor("sb_" + name, list(shape), dt).ap()

            # ---------------- persistent SBUF ----------------
            xsT_s = [sb(f"xsT{i}", [P, N], bf16) for i in range(2)]
            xqT_s = [sb(f"xqT{i}", [P, NQ], bf16) for i in range(2)]
            vq16_s = [sb(f"vq16_{i}", [P, NQ], bf16) for i in range(6)]
            wq_s = [sb(f"wq{i}", [P, HID], bf16) for i in range(2)]
            wk_s = [sb(f"wk{i}", [P, HID], bf16) for i in range(2)]
            wv_s = [sb(f"wv{i}", [P, HID], bf16) for i in range(2)]
            wvec_s = [sb(f"wvec{i}", [P, 2 * HID], bf16) for i in range(2)]
            wo_s = [sb(f"wo{i}", [P, 3 * HID], bf16) for i in range(2)]
            wg_s = [sb(f"wg{i}", [P, HID], bf16) for i in range(4)]
            bq_s = [sb(f"bq{i}", [P, 1], f32) for i in range(2)]
            bk_s = [sb(f"bk{i}", [P, 1], f32) for i in range(2)]
            bg_s = [sb(f"bg{i}", [P, 1], f32) for i in range(2)]
            bo_s = [sb(f"bo{i}", [P, 1], f32) for i in range(6)]
            bvB_s = sb("bvB", [P, HID], f32)
            ones_s = sb("ones", [P, P], bf16)
            kT_s = [sb(f"kT{i}", [P, N], bf16) for i in range(2)]
            qT_s = [sb(f"qT{i}", [P, NQ], bf16) for i in range(2)]
            vall_s = [sb(f"vall{t}", [P, NH * P], bf16) for t in range(NKT)]
            dot_s = [sb(f"dot{j}", [P, NQ], bf16) for j in range(2)]
            norm_s = [sb(f"norm{j}", [P, NQ], bf16) for j in range(2)]
            gate_s = [sb(f"gate{j}", [P, NQ], f32) for j in range(2)]
            xout_s = [sb(f"xout{j}", [P, NQ], bf16) for j in range(2)]
            vaG_s = [[sb(f"vaG{c}_{j}", [P, NQ], f32) for j in range(2)]
                     for c in range(3)]

            dma = nc.sync.dma_start

            # ---------------- input DMAs ----------------
            for i in range(2):
                dma(out=xsT_s[i], in_=xsT[i * P:(i + 1) * P, :])
                dma(out=xqT_s[i], in_=xqT[i * P:(i + 1) * P, :])
                dma(out=wq_s[i], in_=wqT[i * P:(i + 1) * P, :])
                dma(out=wk_s[i], in_=wkT[i * P:(i + 1) * P, :])
                dma(out=wv_s[i], in_=wvT[i * P:(i + 1) * P, :])
                dma(out=wvec_s[i], in_=wvecT[i * P:(i + 1) * P, :])
                dma(out=wo_s[i], in_=woT[i * P:(i + 1) * P, :])
                dma(out=bq_s[i], in_=bq[i * P:(i + 1) * P, :])
                dma(out=bk_s[i], in_=bk[i * P:(i + 1) * P, :])
                dma(out=bg_s[i], in_=bg[i * P:(i + 1) * P, :])
            for i in range(4):
                dma(out=wg_s[i], in_=wgT[i * P:(i + 1) * P, :])
            for i in range(6):
                dma(out=bo_s[i], in_=bo[i * P:(i + 1) * P, :])
                dma(out=vq16_s[i], in_=vq16[i * P:(i + 1) * P, :])
            dma(out=bvB_s, in_=bvB)
            dma(out=ones_s, in_=ones)

            # ---------------- Phase A: projections ----------------
            with tc.tile_pool(name="psA", bufs=3, space="PSUM") as psA, \
                 tc.tile_pool(name="vkvp", bufs=3) as vkvp, \
                 tc.tile_pool(name="vpp", bufs=2) as vpp, \
                 tc.tile_pool(name="tmpp", bufs=2) as tmpp:

                # k^T = Wk @ xs^T   (+bk), bf16, [256, 2048]
                for i in range(2):
                    for j in range(4):
                        ps = psA.tile([P, 512], f32, tag="psA", name="psk")
                        for ic in range(2):
                            nc.tensor.matmul(ps, wk_s[ic][:, ts(i, P)],
                                             xsT_s[ic][:, ts(j, 512)],
                                             start=(ic == 0), stop=(ic == 1))
                        nc.any.tensor_scalar(out=kT_s[i][:, ts(j, 512)], in0=ps,
                                             scalar1=bk_s[i], scalar2=None,
                                             op0=OP.add)
                # q^T = (Wq @ xq^T + bq) * SCALE, bf16, [256, 1024]
                for i in range(2):
                    for j in range(2):
                        ps = psA.tile([P, 512], f32, tag="psA", name="psq")
                        for ic in range(2):
                            nc.tensor.matmul(ps, wq_s[ic][:, ts(i, P)],
                                             xqT_s[ic][:, ts(j, 512)],
                                             start=(ic == 0), stop=(ic == 1))
                        nc.any.tensor_scalar(out=qT_s[i][:, ts(j, 512)], in0=ps,
                                             scalar1=bq_s[i], scalar2=SCALE,
                                             op0=OP.add, op1=OP.mult)

                # v token-major + v_all assembly
                for t in range(NKT):
                    vk = vkvp.tile([P, 3 * HID], bf16, tag="vk", name="vk")
                    dma(out=vk, in_=vkv[t * P:(t + 1) * P, :])
                    ps = psA.tile([P, HID], f32, tag="psV", name="psv")
                    for ic in range(2):
                        nc.tensor.matmul(ps, xsT_s[ic][:, ts(t, P)], wv_s[ic],
                                         start=(ic == 0), stop=(ic == 1))
                    va3 = vall_s[t].rearrange("p (h d) -> p h d", h=NH)
                    ps3 = ps.rearrange("p (h d) -> p h d", d=HD)
                    bv3 = bvB_s.rearrange("p (h d) -> p h d", d=HD)
                    nc.any.tensor_tensor(out=va3[:, :, 0:HD], in0=ps3, in1=bv3,
                                         op=OP.add)
                    vk4 = vk.rearrange("p (c h d) -> p c h d", c=3, d=HD)
                    for c in range(3):
                        nc.any.tensor_copy(
                            out=va3[:, :, HD + c * HD: 2 * HD + c * HD],
                            in_=vk4[:, c])

                # vec_proj (query half) + vec_dot
                for c in range(3):
                    vp = []
                    for o in range(4):
                        vpt = vpp.tile([P, NQ], bf16, tag=f"vp{o}",
                                       name=f"vp{o}")
                        for n in range(2):
                            ps = psA.tile([P, 512], f32, tag="psA", name="psp")
                            for ic in range(2):
                                nc.tensor.matmul(
                                    ps, wvec_s[ic][:, ts(o, P)],
                                    vq16_s[2 * c + ic][:, ts(n, 512)],
                                    start=(ic == 0), stop=(ic == 1))
                            nc.any.tensor_copy(out=vpt[:, ts(n, 512)], in_=ps)
                        vp.append(vpt)
                    for j in range(2):
                        if c == 0:
                            nc.any.tensor_tensor(out=dot_s[j], in0=vp[j],
                                                 in1=vp[2 + j], op=OP.mult)
                        else:
                            m = tmpp.tile([P, NQ], bf16, tag="dtmp",
                                          name="dtmp")
                            nc.any.tensor_tensor(out=m, in0=vp[j],
                                                 in1=vp[2 + j], op=OP.mult)
                            nc.any.tensor_tensor(out=dot_s[j], in0=dot_s[j],
                                                 in1=m, op=OP.add)

                # vec_norm (query half)
                for j in range(2):
                    nt = tmpp.tile([P, NQ], bf16, tag="ntmp", name="ntmp")
                    nc.any.tensor_tensor(out=nt, in0=vq16_s[j], in1=vq16_s[j],
                                         op=OP.mult)
                    for c in (1, 2):
                        m = tmpp.tile([P, NQ], bf16, tag="ntmp2", name="ntmp2")
                        nc.any.tensor_tensor(out=m, in0=vq16_s[2 * c + j],
                                             in1=vq16_s[2 * c + j], op=OP.mult)
                        nc.any.tensor_tensor(out=nt, in0=nt, in1=m, op=OP.add)
                    nc.scalar.activation(norm_s[j], nt, AF.Sqrt)

                # gate = sigmoid(Wg_scaled @ [dot; norm] + bg)
                inv_tiles = [dot_s[0], dot_s[1], norm_s[0], norm_s[1]]
                for o in range(2):
                    for n in range(2):
                        ps = psA.tile([P, 512], f32, tag="psA", name="psg")
                        for ic in range(4):
                            nc.tensor.matmul(ps, wg_s[ic][:, ts(o, P)],
                                             inv_tiles[ic][:, ts(n, 512)],
                                             start=(ic == 0), stop=(ic == 3))
                        nc.scalar.activation(gate_s[o][:, ts(n, 512)], ps,
                                             AF.Sigmoid, bias=bg_s[o])

            # ---------------- Phase B: attention ----------------
            # Head-quads j=0 (heads 0-3) and j=1 (heads 4-7). Per (j, qc):
            #  - S^T matmuls row-packed in head pairs into psum_s [128,1024]
            #  - one exp per pair tile
            #  - PV + denominator column-packed (tile_position=(0,32m)) so
            #    head 4j+m lands on partitions 32m..32m+32 of shared psum
            #    accumulators: xo (out_s), va0-2 (vec aggr), dn (softmax den)
            with tc.tile_pool(name="psS", bufs=1, space="PSUM") as psS, \
                 tc.tile_pool(name="psAcc", bufs=1, space="PSUM") as psAcc, \
                 tc.tile_pool(name="expp", bufs=3) as expp, \
                 tc.tile_pool(name="rcpp", bufs=2) as rcpp:
                for j in range(2):
                    for qc in range(2):
                        xo = psAcc.tile([P, 512], f32, tag="xo", name="xo")
                        va = [psAcc.tile([P, 512], f32, tag=f"va{c}",
                                         name=f"va{c}") for c in range(3)]
                        dn = psAcc.tile([P, 512], f32, tag="dn", name="dn")
                        for kt in range(NKT):
                            for pp in range(2):
                                ss = psS.tile([P, 1024], f32, tag="ss",
                                              name="ss")
                                for e in range(2):
                                    r = 2 * pp + e
                                    nc.tensor.matmul(
                                        ss[:, ts(e, 512)],
                                        kT_s[j][32 * r:32 * r + 32, ts(kt, P)],
                                        qT_s[j][32 * r:32 * r + 32,
                                                ts(qc, 512)],
                                        start=True, stop=True,
                                        tile_position=(32 * r, 0))
                                ex = expp.tile([P, 1024], bf16, tag="ex",
                                               name="ex")
                                nc.scalar.activation(ex, ss, AF.Exp)
                                for e in range(2):
                                    m = 2 * pp + e
                                    h = 4 * j + m
                                    exs = ex[:, ts(e, 512)]
                                    st, sp = (kt == 0), (kt == NKT - 1)
                                    nc.tensor.matmul(
                                        xo[32 * m:32 * m + 32, :],
                                        vall_s[kt][:, h * P: h * P + HD],
                                        exs, start=st, stop=sp,
                                        tile_position=(0, 32 * m))
                                    for c in range(3):
                                        nc.tensor.matmul(
                                            va[c][32 * m:32 * m + 32, :],
                                            vall_s[kt][:, h * P + HD + c * HD:
                                                       h * P + 2 * HD + c * HD],
                                            exs, start=st, stop=sp,
                                            tile_position=(0, 32 * m))
                                    nc.tensor.matmul(
                                        dn[32 * m:32 * m + 32, :],
                                        ones_s[:, 0:HD],
                                        exs, start=st, stop=sp,
                                        tile_position=(0, 32 * m))
                        rc = rcpp.tile([P, 512], f32, tag="rc", name="rc")
                        nc.vector.reciprocal_approx_fast(out=rc, in_=dn)
                        nc.any.tensor_tensor(out=xout_s[j][:, ts(qc, 512)],
                                             in0=xo, in1=rc, op=OP.mult)
                        for c in range(3):
                            nc.any.tensor_tensor(
                                out=vaG_s[c][j][:, ts(qc, 512)],
                                in0=va[c], in1=rc, op=OP.mult)

            # ---------------- epilogue ----------------
            with tc.tile_pool(name="psE", bufs=2, space="PSUM") as psE, \
                 tc.tile_pool(name="outp", bufs=2) as outp, \
                 tc.tile_pool(name="vqp", bufs=2) as vqp:
                for j in range(2):
                    for n in range(2):
                        pso = [psE.tile([P, 512], f32, tag=f"po{k}",
                                        name=f"po{k}") for k in range(3)]
                        for k in range(3):
                            o_idx = 2 * k + j
                            for ic in range(2):
                                nc.tensor.matmul(pso[k],
                                                 wo_s[ic][:, ts(o_idx, P)],
                                                 xout_s[ic][:, ts(n, 512)],
                                                 start=(ic == 0),
                                                 stop=(ic == 1))
                        t1 = outp.tile([P, 512], f32, tag="t1", name="t1")
                        nc.vector.scalar_tensor_tensor(
                            out=t1, in0=pso[0], scalar=bo_s[j],
                            in1=dot_s[j][:, ts(n, 512)],
                            op0=OP.add, op1=OP.mult)
                        t2 = outp.tile([P, 512], f32, tag="t2", name="t2")
                        nc.vector.scalar_tensor_tensor(
                            out=t2, in0=pso[1], scalar=bo_s[2 + j],
                            in1=norm_s[j][:, ts(n, 512)],
                            op0=OP.add, op1=OP.mult)
                        nc.any.tensor_tensor(out=t1, in0=t1, in1=t2, op=OP.add)
                        xu = outp.tile([P, 512], f32, tag="xu", name="xu")
                        nc.vector.scalar_tensor_tensor(
                            out=xu, in0=pso[2], scalar=bo_s[4 + j], in1=t1,
                            op0=OP.add, op1=OP.add)
                        dma(out=out[j * P:(j + 1) * P, ts(n, 512)], in_=xu)

                for c in range(3):
                    for j in range(2):
                        for n in range(2):
                            vq = vqp.tile([P, 512], f32, tag="vq", name="vq")
                            dma(out=vq, in_=vq32[(2 * c + j) * P:
                                                 (2 * c + j + 1) * P,
                                                 ts(n, 512)])
                            t = outp.tile([P, 512], f32, tag="vc", name="vc")
                            nc.any.tensor_tensor(out=t,
                                                 in0=gate_s[j][:, ts(n, 512)],
                                                 in1=vaG_s[c][j][:, ts(n, 512)],
                                                 op=OP.mult)
                            nc.any.tensor_tensor(out=t, in0=t, in1=vq,
                                                 op=OP.add)
                            r0_ = (1 + c) * HID + j * P
                            dma(out=out[r0_:r0_ + P, ts(n, 512)], in_=t)

    nc.compile()
    return nc


def _get_nc():
    if "nc" not in _CACHE:
        _CACHE["nc"] = _build_nc()
    return _CACHE["nc"]


def _make_in_maps(inputs):
    x = np.asarray(inputs["x"], np.float32)
    Wq = np.asarray(inputs["Wq"], np.float32)
    Wk = np.asarray(inputs["Wk"], np.float32)
    Wv = np.asarray(inputs["Wv"], np.float32)
    Wvec = np.asarray(inputs["Wvec"], np.float32)
    Wo = np.asarray(inputs["Wo"], np.float32)
    Wg = np.asarray(inputs["Wg"], np.float32)
    bq = np.asarray(inputs["bq"], np.float32)
    bk = np.asarray(inputs["bk"], np.float32)
    bv = np.asarray(inputs["bv"], np.float32)
    bo = np.asarray(inputs["bo"], np.float32)
    bg = np.asarray(inputs["bg"], np.float32)
    a_d = float(np.asarray(inputs["alpha_dot"]))
    a_n = float(np.asarray(inputs["alpha_norm"]))

    wgT = Wg.T.copy()
    wgT[:HID, :] *= a_d
    wgT[HID:, :] *= a_n

    common = {
        "wqT": np.ascontiguousarray(Wq.T).astype(BF),
        "wkT": np.ascontiguousarray(Wk.T).astype(BF),
        "wvT": np.ascontiguousarray(Wv.T).astype(BF),
        "wvecT": np.ascontiguousarray(Wvec.T).astype(BF),
        "woT": np.ascontiguousarray(Wo.T).astype(BF),
        "wgT": np.ascontiguousarray(wgT).astype(BF),
        "bq": np.ascontiguousarray(bq.reshape(HID, 1)),
        "bk": np.ascontiguousarray(bk.reshape(HID, 1)),
        "bg": np.ascontiguousarray(bg.reshape(HID, 1)),
        "bo": np.ascontiguousarray(bo.reshape(3 * HID, 1)),
        "bvB": np.ascontiguousarray(np.broadcast_to(bv, (128, HID))),
        "ones": np.ones((128, 128), BF),
    }

    in_maps = []
    for core in range(8):
        b, qh = core // 2, core % 2
        qs = slice(qh * NQ, (qh + 1) * NQ)
        xsT = np.ascontiguousarray(x[b, :, 0, :].T)
        vq = x[b, qs, 1:, :].transpose(1, 2, 0).reshape(3 * HID, NQ)
        m = dict(common)
        m["xsT"] = xsT.astype(BF)
        m["xqT"] = np.ascontiguousarray(xsT[:, qs]).astype(BF)
        m["vq32"] = np.ascontiguousarray(vq)
        m["vq16"] = np.ascontiguousarray(vq).astype(BF)
        m["vkv"] = np.ascontiguousarray(
            x[b, :, 1:, :].reshape(N, 3 * HID)).astype(BF)
        in_maps.append(m)
    return in_maps


def _gather(results):
    x_final = np.empty((B, N, 4, HID), np.float32)
    for core, res in enumerate(results):
        b, qh = core // 2, core % 2
        qs = slice(qh * NQ, (qh + 1) * NQ)
        o = res["out"]                       # [1024 ch, 1024 q]
        for c in range(4):
            x_final[b, qs, c, :] = o[c * HID:(c + 1) * HID, :].T
    return x_final


def _run(inputs, trace=False):
    from concourse.bass_utils import run_bass_kernel_spmd
    nc = _get_nc()
    in_maps = _make_in_maps(inputs)
    res = run_bass_kernel_spmd(nc, in_maps, core_ids=list(range(8)),
                               trace=trace)
    return _gather(res.results), res


def kernel(**inputs):
    out, _ = _run(inputs, trace=False)
    return out


def _install_trace_hook():
    try:
        import antenv.axon_hooks as ah
    except ModuleNotFoundError:
        import importlib.util
        spec = importlib.util.spec_from_file_location(
            "antenv.axon_hooks", "/opt/trn_rl_repo/antenv/axon_hooks.py")
        ah = importlib.util.module_from_spec(spec)
        sys.modules["antenv.axon_hooks"] = ah
        spec.loader.exec_module(ah)
    if ah.get_axon_ntff_profile_hook() is None:
        from trn_agent_boot.trn_boot import _ntff_profile_via_ctypes
        ah.set_axon_ntff_profile_hook(
            _ntff_profile_via_ctypes("/opt/axon/libaxon_pjrt.so"))
    # avoid the cloud-bucket artifact upload in the trace path
    import concourse.bass_utils as bu
    bu.upload_artifacts = lambda tmpdir: tmpdir


def run_traced(inputs, tmpdir=None):
    _install_trace_hook()
    from concourse.bass_utils import run_bass_kernel_spmd
    nc = _get_nc()
    in_maps = _make_in_maps(inputs)
    res = run_bass_kernel_spmd(nc, in_maps, core_ids=list(range(8)),
                               trace=True, tmpdir=tmpdir)
    return _gather(res.results), res


# revision 8
# speedup vs baseline: 1.5121x; 1.5121x over previous
"""Trainium2 Bass kernel for EquivariantSelfAttention (B=4, N=2048, HID=256, 8 heads).

Sharding: 8 cores = 4 batches x 2 query-halves. Each core computes full
attention for one batch over its 1024 queries (all 2048 keys), plus the
per-token epilogue, fully locally (no collectives).

Device layout is channel-major ("transposed"): all transposes are done on the
host (numpy) during shard prep / output gather, so the device only runs
matmuls + elementwise work on [channels, tokens] tiles.
"""

import sys

if "/opt/trn_rl_repo" not in sys.path:
    sys.path.insert(0, "/opt/trn_rl_repo")

import numpy as np
import ml_dtypes

B, N, HID, NH, HD = 4, 2048, 256, 8, 32
NQ = N // 2          # queries per core
NKT = N // 128       # key tiles
SCALE = float(1.0 / np.sqrt(HD))
BF = ml_dtypes.bfloat16

_CACHE = {}


def _build_nc():
    import concourse.bass as bass
    import concourse.mybir as mybir
    import concourse.tile as tile
    from concourse import bacc
    from concourse.bass import ts

    f32 = mybir.dt.float32
    bf16 = mybir.dt.bfloat16
    AF = mybir.ActivationFunctionType
    OP = mybir.AluOpType
    P = 128

    nc = bacc.Bacc("TRN2", target_bir_lowering=False, debug=False,
                   enable_asserts=False, num_devices=8)

    def din(name, shape, dt):
        return nc.dram_tensor(name, shape, dt, kind="ExternalInput").ap()

    xsT = din("xsT", [2 * P, N], bf16)        # x_scalar^T, all tokens
    xqT = din("xqT", [2 * P, NQ], bf16)       # x_scalar^T, query half
    vq32 = din("vq32", [6 * P, NQ], f32)      # vec^T (query half), f32
    vq16 = din("vq16", [6 * P, NQ], bf16)     # vec^T (query half), bf16
    vkv = din("vkv", [N, 3 * HID], bf16)      # vec token-major, all tokens
    wqT = din("wqT", [HID, HID], bf16)
    wkT = din("wkT", [HID, HID], bf16)
    wvT = din("wvT", [HID, HID], bf16)
    wvecT = din("wvecT", [HID, 2 * HID], bf16)
    woT = din("woT", [HID, 3 * HID], bf16)
    wgT = din("wgT", [2 * HID, HID], bf16)    # alpha-folded
    bq = din("bq", [HID, 1], f32)
    bk = din("bk", [HID, 1], f32)
    bvB = din("bvB", [P, HID], f32)           # bv broadcast over partitions
    bg = din("bg", [HID, 1], f32)
    bo = din("bo", [3 * HID, 1], f32)
    ones = din("ones", [P, P], bf16)
    out = nc.dram_tensor("out", [4 * HID, NQ], f32, kind="ExternalOutput").ap()

    with tile.TileContext(nc) as tc:
        from contextlib import ExitStack
        with ExitStack() as ctx:
            def sb(name, shape, dt):
                return nc.alloc_sbuf_tensor("sb_" + name, list(shape), dt).ap()

            # ---------------- persistent SBUF ----------------
            xsT_s = [sb(f"xsT{i}", [P, N], bf16) for i in range(2)]
            xqT_s = [sb(f"xqT{i}", [P, NQ], bf16) for i in range(2)]
            vq16_s = [sb(f"vq16_{i}", [P, NQ], bf16) for i in range(6)]
            wq_s = [sb(f"wq{i}", [P, HID], bf16) for i in range(2)]
            wk_s = [sb(f"wk{i}", [P, HID], bf16) for i in range(2)]
            wv_s = [sb(f"wv{i}", [P, HID], bf16) for i in range(2)]
            wvec_s = [sb(f"wvec{i}", [P, 2 * HID], bf16) for i in range(2)]
            wo_s = [sb(f"wo{i}", [P, 3 * HID], bf16) for i in range(2)]
            wg_s = [sb(f"wg{i}", [P, HID], bf16) for i in range(4)]
            bq_s = [sb(f"bq{i}", [P, 1], f32) for i in range(2)]
            bk_s = [sb(f"bk{i}", [P, 1], f32) for i in range(2)]
            bg_s = [sb(f"bg{i}", [P, 1], f32) for i in range(2)]
            bo_s = [sb(f"bo{i}", [P, 1], f32) for i in range(6)]
            bvB_s = sb("bvB", [P, HID], f32)
            ones_s = sb("ones", [P, P], bf16)
            kT_s = [sb(f"kT{i}", [P, N], bf16) for i in range(2)]
            qT_s = [sb(f"qT{i}", [P, NQ], bf16) for i in range(2)]
            vall_s = [sb(f"vall{t}", [P, NH * P], bf16) for t in range(NKT)]
            dot_s = [sb(f"dot{j}", [P, NQ], bf16) for j in range(2)]
            norm_s = [sb(f"norm{j}", [P, NQ], bf16) for j in range(2)]
            gate_s = [sb(f"gate{j}", [P, NQ], f32) for j in range(2)]
            xout_s = [sb(f"xout{j}", [P, NQ], bf16) for j in range(2)]
            vaG_s = [[sb(f"vaG{c}_{j}", [P, NQ], f32) for j in range(2)]
                     for c in range(3)]

            dma = nc.sync.dma_start

            # ---------------- input DMAs ----------------
            for i in range(2):
                dma(out=xsT_s[i], in_=xsT[i * P:(i + 1) * P, :])
                dma(out=xqT_s[i], in_=xqT[i * P:(i + 1) * P, :])
                dma(out=wq_s[i], in_=wqT[i * P:(i + 1) * P, :])
                dma(out=wk_s[i], in_=wkT[i * P:(i + 1) * P, :])
                dma(out=wv_s[i], in_=wvT[i * P:(i + 1) * P, :])
                dma(out=wvec_s[i], in_=wvecT[i * P:(i + 1) * P, :])
                dma(out=wo_s[i], in_=woT[i * P:(i + 1) * P, :])
                dma(out=bq_s[i], in_=bq[i * P:(i + 1) * P, :])
                dma(out=bk_s[i], in_=bk[i * P:(i + 1) * P, :])
                dma(out=bg_s[i], in_=bg[i * P:(i + 1) * P, :])
            for i in range(4):
                dma(out=wg_s[i], in_=wgT[i * P:(i + 1) * P, :])
            for i in range(6):
                dma(out=bo_s[i], in_=bo[i * P:(i + 1) * P, :])
                dma(out=vq16_s[i], in_=vq16[i * P:(i + 1) * P, :])
            dma(out=bvB_s, in_=bvB)
            dma(out=ones_s, in_=ones)

            # ---------------- Phase A: projections ----------------
            with tc.tile_pool(name="psA", bufs=3, space="PSUM") as psA, \
                 tc.tile_pool(name="vkvp", bufs=3) as vkvp, \
                 tc.tile_pool(name="vpp", bufs=2) as vpp, \
                 tc.tile_pool(name="tmpp", bufs=2) as tmpp:

                # k^T = Wk @ xs^T   (+bk), bf16, [256, 2048]
                for i in range(2):
                    for j in range(4):
                        ps = psA.tile([P, 512], f32, tag="psA", name="psk")
                        for ic in range(2):
                            nc.tensor.matmul(ps, wk_s[ic][:, ts(i, P)],
                                             xsT_s[ic][:, ts(j, 512)],
                                             start=(ic == 0), stop=(ic == 1))
                        nc.any.tensor_scalar(out=kT_s[i][:, ts(j, 512)], in0=ps,
                                             scalar1=bk_s[i], scalar2=None,
                                             op0=OP.add)
                # q^T = (Wq @ xq^T + bq) * SCALE, bf16, [256, 1024]
                for i in range(2):
                    for j in range(2):
                        ps = psA.tile([P, 512], f32, tag="psA", name="psq")
                        for ic in range(2):
                            nc.tensor.matmul(ps, wq_s[ic][:, ts(i, P)],
                                             xqT_s[ic][:, ts(j, 512)],
                                             start=(ic == 0), stop=(ic == 1))
                        nc.any.tensor_scalar(out=qT_s[i][:, ts(j, 512)], in0=ps,
                                             scalar1=bq_s[i], scalar2=SCALE,
                                             op0=OP.add, op1=OP.mult)

                # v token-major + v_all assembly
                for t in range(NKT):
                    vk = vkvp.tile([P, 3 * HID], bf16, tag="vk", name="vk")
                    dma(out=vk, in_=vkv[t * P:(t + 1) * P, :])
                    ps = psA.tile([P, HID], f32, tag="psV", name="psv")
                    for ic in range(2):
                        nc.tensor.matmul(ps, xsT_s[ic][:, ts(t, P)], wv_s[ic],
                                         start=(ic == 0), stop=(ic == 1))
                    va3 = vall_s[t].rearrange("p (h d) -> p h d", h=NH)
                    ps3 = ps.rearrange("p (h d) -> p h d", d=HD)
                    bv3 = bvB_s.rearrange("p (h d) -> p h d", d=HD)
                    nc.vector.tensor_tensor(out=va3[:, :, 0:HD], in0=ps3,
                                            in1=bv3, op=OP.add)
                    vk4 = vk.rearrange("p (c h d) -> p c h d", c=3, d=HD)
                    for c in range(3):
                        nc.vector.tensor_copy(
                            va3[:, :, HD + c * HD: 2 * HD + c * HD],
                            vk4[:, c])

                # vec_proj (query half) + vec_dot
                for c in range(3):
                    vp = []
                    for o in range(4):
                        vpt = vpp.tile([P, NQ], bf16, tag=f"vp{o}",
                                       name=f"vp{o}")
                        for n in range(2):
                            ps = psA.tile([P, 512], f32, tag="psA", name="psp")
                            for ic in range(2):
                                nc.tensor.matmul(
                                    ps, wvec_s[ic][:, ts(o, P)],
                                    vq16_s[2 * c + ic][:, ts(n, 512)],
                                    start=(ic == 0), stop=(ic == 1))
                            nc.any.tensor_copy(out=vpt[:, ts(n, 512)], in_=ps)
                        vp.append(vpt)
                    for j in range(2):
                        if c == 0:
                            nc.any.tensor_tensor(out=dot_s[j], in0=vp[j],
                                                 in1=vp[2 + j], op=OP.mult)
                        else:
                            m = tmpp.tile([P, NQ], bf16, tag="dtmp",
                                          name="dtmp")
                            nc.any.tensor_tensor(out=m, in0=vp[j],
                                                 in1=vp[2 + j], op=OP.mult)
                            nc.any.tensor_tensor(out=dot_s[j], in0=dot_s[j],
                                                 in1=m, op=OP.add)

                # vec_norm (query half)
                for j in range(2):
                    nt = tmpp.tile([P, NQ], bf16, tag="ntmp", name="ntmp")
                    nc.any.tensor_tensor(out=nt, in0=vq16_s[j], in1=vq16_s[j],
                                         op=OP.mult)
                    for c in (1, 2):
                        m = tmpp.tile([P, NQ], bf16, tag="ntmp2", name="ntmp2")
                        nc.any.tensor_tensor(out=m, in0=vq16_s[2 * c + j],
                                             in1=vq16_s[2 * c + j], op=OP.mult)
                        nc.any.tensor_tensor(out=nt, in0=nt, in1=m, op=OP.add)
                    nc.scalar.activation(norm_s[j], nt, AF.Sqrt)

                # gate = sigmoid(Wg_scaled @ [dot; norm] + bg)
                inv_tiles = [dot_s[0], dot_s[1], norm_s[0], norm_s[1]]
                for o in range(2):
                    for n in range(2):
                        ps = psA.tile([P, 512], f32, tag="psA", name="psg")
                        for ic in range(4):
                            nc.tensor.matmul(ps, wg_s[ic][:, ts(o, P)],
                                             inv_tiles[ic][:, ts(n, 512)],
                                             start=(ic == 0), stop=(ic == 3))
                        nc.scalar.activation(gate_s[o][:, ts(n, 512)], ps,
                                             AF.Sigmoid, bias=bg_s[o])

            # ---------------- Phase B: attention ----------------
            # Head-quads j=0 (heads 0-3) and j=1 (heads 4-7). Per (j, qc):
            #  - S^T matmuls row-packed in head pairs into psum_s [128,1024]
            #  - one exp per pair tile
            #  - PV + denominator column-packed (tile_position=(0,32m)) so
            #    head 4j+m lands on partitions 32m..32m+32 of shared psum
            #    accumulators: xo (out_s), va0-2 (vec aggr), dn (softmax den)
            with tc.tile_pool(name="psS", bufs=1, space="PSUM") as psS, \
                 tc.tile_pool(name="psAcc", bufs=1, space="PSUM") as psAcc, \
                 tc.tile_pool(name="expp", bufs=3) as expp, \
                 tc.tile_pool(name="rcpp", bufs=2) as rcpp:
                for j in range(2):
                    for qc in range(2):
                        xo = psAcc.tile([P, 512], f32, tag="xo", name="xo")
                        va = [psAcc.tile([P, 512], f32, tag=f"va{c}",
                                         name=f"va{c}") for c in range(3)]
                        dn = psAcc.tile([P, 512], f32, tag="dn", name="dn")

                        def emit_pv(kt, pp, ex):
                            for e in range(2):
                                m = 2 * pp + e
                                h = 4 * j + m
                                exs = ex[:, ts(e, 512)]
                                st = (kt == 0)
                                sp = (kt == NKT - 1)
                                nc.tensor.matmul(
                                    xo[32 * m:32 * m + 32, :],
                                    vall_s[kt][:, h * P: h * P + HD],
                                    exs, start=st, stop=sp,
                                    tile_position=(0, 32 * m))
                                for c in range(3):
                                    nc.tensor.matmul(
                                        va[c][32 * m:32 * m + 32, :],
                                        vall_s[kt][:, h * P + HD + c * HD:
                                                   h * P + 2 * HD + c * HD],
                                        exs, start=st, stop=sp,
                                        tile_position=(0, 32 * m))
                                nc.tensor.matmul(
                                    dn[32 * m:32 * m + 32, :],
                                    ones_s[:, 0:HD],
                                    exs, start=st, stop=sp,
                                    tile_position=(0, 32 * m))

                        pending = None
                        for kt in range(NKT):
                            for pp in range(2):
                                ss = psS.tile([P, 1024], f32, tag="ss",
                                              name="ss")
                                for e in range(2):
                                    r = 2 * pp + e
                                    nc.tensor.matmul(
                                        ss[:, ts(e, 512)],
                                        kT_s[j][32 * r:32 * r + 32, ts(kt, P)],
                                        qT_s[j][32 * r:32 * r + 32,
                                                ts(qc, 512)],
                                        start=True, stop=True,
                                        tile_position=(32 * r, 0))
                                ex = expp.tile([P, 1024], bf16, tag="ex",
                                               name="ex")
                                nc.scalar.activation(ex, ss, AF.Exp)
                                if pending is not None:
                                    emit_pv(*pending)
                                pending = (kt, pp, ex)
                        emit_pv(*pending)
                        rc = rcpp.tile([P, 512], f32, tag="rc", name="rc")
                        nc.vector.reciprocal_approx_fast(out=rc, in_=dn)
                        nc.vector.tensor_tensor(out=xout_s[j][:, ts(qc, 512)],
                                                in0=xo, in1=rc, op=OP.mult)
                        for c in range(3):
                            nc.vector.tensor_tensor(
                                out=vaG_s[c][j][:, ts(qc, 512)],
                                in0=va[c], in1=rc, op=OP.mult)

            # ---------------- epilogue ----------------
            with tc.tile_pool(name="psE", bufs=2, space="PSUM") as psE, \
                 tc.tile_pool(name="outp", bufs=2) as outp, \
                 tc.tile_pool(name="vqp", bufs=2) as vqp:
                for j in range(2):
                    for n in range(2):
                        pso = [psE.tile([P, 512], f32, tag=f"po{k}",
                                        name=f"po{k}") for k in range(3)]
                        for k in range(3):
                            o_idx = 2 * k + j
                            for ic in range(2):
                                nc.tensor.matmul(pso[k],
                                                 wo_s[ic][:, ts(o_idx, P)],
                                                 xout_s[ic][:, ts(n, 512)],
                                                 start=(ic == 0),
                                                 stop=(ic == 1))
                        t1 = outp.tile([P, 512], f32, tag="t1", name="t1")
                        nc.vector.scalar_tensor_tensor(
                            out=t1, in0=pso[0], scalar=bo_s[j],
                            in1=dot_s[j][:, ts(n, 512)],
                            op0=OP.add, op1=OP.mult)
                        t2 = outp.tile([P, 512], f32, tag="t2", name="t2")
                        nc.vector.scalar_tensor_tensor(
                            out=t2, in0=pso[1], scalar=bo_s[2 + j],
                            in1=norm_s[j][:, ts(n, 512)],
                            op0=OP.add, op1=OP.mult)
                        nc.any.tensor_tensor(out=t1, in0=t1, in1=t2, op=OP.add)
                        xu = outp.tile([P, 512], f32, tag="xu", name="xu")
                        nc.vector.scalar_tensor_tensor(
                            out=xu, in0=pso[2], scalar=bo_s[4 + j], in1=t1,
                            op0=OP.add, op1=OP.add)
                        dma(out=out[j * P:(j + 1) * P, ts(n, 512)], in_=xu)

                for c in range(3):
                    for j in range(2):
                        for n in range(2):
                            vq = vqp.tile([P, 512], f32, tag="vq", name="vq")
                            dma(out=vq, in_=vq32[(2 * c + j) * P:
                                                 (2 * c + j + 1) * P,
                                                 ts(n, 512)])
                            t = outp.tile([P, 512], f32, tag="vc", name="vc")
                            nc.any.tensor_tensor(out=t,
                                                 in0=gate_s[j][:, ts(n, 512)],
                                                 in1=vaG_s[c][j][:, ts(n, 512)],
                                                 op=OP.mult)
                            nc.any.tensor_tensor(out=t, in0=t, in1=vq,
                                                 op=OP.add)
                            r0_ = (1 + c) * HID + j * P
                            dma(out=out[r0_:r0_ + P, ts(n, 512)], in_=t)

    nc.compile()
    return nc


def _get_nc():
    if "nc" not in _CACHE:
        _CACHE["nc"] = _build_nc()
    return _CACHE["nc"]


def _make_in_maps(inputs):
    x = np.asarray(inputs["x"], np.float32)
    Wq = np.asarray(inputs["Wq"], np.float32)
    Wk = np.asarray(inputs["Wk"], np.float32)
    Wv = np.asarray(inputs["Wv"], np.float32)
    Wvec = np.asarray(inputs["Wvec"], np.float32)
    Wo = np.asarray(inputs["Wo"], np.float32)
    Wg = np.asarray(inputs["Wg"], np.float32)
    bq = np.asarray(inputs["bq"], np.float32)
    bk = np.asarray(inputs["bk"], np.float32)
    bv = np.asarray(inputs["bv"], np.float32)
    bo = np.asarray(inputs["bo"], np.float32)
    bg = np.asarray(inputs["bg"], np.float32)
    a_d = float(np.asarray(inputs["alpha_dot"]))
    a_n = float(np.asarray(inputs["alpha_norm"]))

    wgT = Wg.T.copy()
    wgT[:HID, :] *= a_d
    wgT[HID:, :] *= a_n

    common = {
        "wqT": np.ascontiguousarray(Wq.T).astype(BF),
        "wkT": np.ascontiguousarray(Wk.T).astype(BF),
        "wvT": np.ascontiguousarray(Wv.T).astype(BF),
        "wvecT": np.ascontiguousarray(Wvec.T).astype(BF),
        "woT": np.ascontiguousarray(Wo.T).astype(BF),
        "wgT": np.ascontiguousarray(wgT).astype(BF),
        "bq": np.ascontiguousarray(bq.reshape(HID, 1)),
        "bk": np.ascontiguousarray(bk.reshape(HID, 1)),
        "bg": np.ascontiguousarray(bg.reshape(HID, 1)),
        "bo": np.ascontiguousarray(bo.reshape(3 * HID, 1)),
        "bvB": np.ascontiguousarray(np.broadcast_to(bv, (128, HID))),
        "ones": np.ones((128, 128), BF),
    }

    in_maps = []
    for core in range(8):
        b, qh = core // 2, core % 2
        qs = slice(qh * NQ, (qh + 1) * NQ)
        xsT = np.ascontiguousarray(x[b, :, 0, :].T)
        vq = x[b, qs, 1:, :].transpose(1, 2, 0).reshape(3 * HID, NQ)
        m = dict(common)
        m["xsT"] = xsT.astype(BF)
        m["xqT"] = np.ascontiguousarray(xsT[:, qs]).astype(BF)
        m["vq32"] = np.ascontiguousarray(vq)
        m["vq16"] = np.ascontiguousarray(vq).astype(BF)
        m["vkv"] = np.ascontiguousarray(
            x[b, :, 1:, :].reshape(N, 3 * HID)).astype(BF)
        in_maps.append(m)
    return in_maps


def _gather(results):
    x_final = np.empty((B, N, 4, HID), np.float32)
    for core, res in enumerate(results):
        b, qh = core // 2, core % 2
        qs = slice(qh * NQ, (qh + 1) * NQ)
        o = res["out"]                       # [1024 ch, 1024 q]
        for c in range(4):
            x_final[b, qs, c, :] = o[c * HID:(c + 1) * HID, :].T
    return x_final


def _run(inputs, trace=False):
    from concourse.bass_utils import run_bass_kernel_spmd
    nc = _get_nc()
    in_maps = _make_in_maps(inputs)
    res = run_bass_kernel_spmd(nc, in_maps, core_ids=list(range(8)),
                               trace=trace)
    return _gather(res.results), res


def kernel(**inputs):
    out, _ = _run(inputs, trace=False)
    return out


def _install_trace_hook():
    try:
        import antenv.axon_hooks as ah
    except ModuleNotFoundError:
        import importlib.util
        spec = importlib.util.spec_from_file_location(
            "antenv.axon_hooks", "/opt/trn_rl_repo/antenv/axon_hooks.py")
        ah = importlib.util.module_from_spec(spec)
        sys.modules["antenv.axon_hooks"] = ah
        spec.loader.exec_module(ah)
    if ah.get_axon_ntff_profile_hook() is None:
        from trn_agent_boot.trn_boot import _ntff_profile_via_ctypes
        ah.set_axon_ntff_profile_hook(
            _ntff_profile_via_ctypes("/opt/axon/libaxon_pjrt.so"))
    # avoid the cloud-bucket artifact upload in the trace path
    import concourse.bass_utils as bu
    bu.upload_artifacts = lambda tmpdir: tmpdir


def run_traced(inputs, tmpdir=None):
    _install_trace_hook()
    from concourse.bass_utils import run_bass_kernel_spmd
    nc = _get_nc()
    in_maps = _make_in_maps(inputs)
    res = run_bass_kernel_spmd(nc, in_maps, core_ids=list(range(8)),
                               trace=True, tmpdir=tmpdir)
    return _gather(res.results), res


# revision 9
# speedup vs baseline: 1.5724x; 1.0399x over previous
"""Trainium2 Bass kernel for EquivariantSelfAttention (B=4, N=2048, HID=256, 8 heads).

Sharding: 8 cores = 4 batches x 2 query-halves. Each core computes full
attention for one batch over its 1024 queries (all 2048 keys), plus the
per-token epilogue, fully locally (no collectives).

Device layout is channel-major ("transposed"): all transposes are done on the
host (numpy) during shard prep / output gather, so the device only runs
matmuls + elementwise work on [channels, tokens] tiles.
"""

import sys

if "/opt/trn_rl_repo" not in sys.path:
    sys.path.insert(0, "/opt/trn_rl_repo")

import numpy as np
import ml_dtypes

B, N, HID, NH, HD = 4, 2048, 256, 8, 32
NQ = N // 2          # queries per core
NKT = N // 128       # key tiles
SCALE = float(1.0 / np.sqrt(HD))
BF = ml_dtypes.bfloat16

_CACHE = {}


def _build_nc():
    import concourse.bass as bass
    import concourse.mybir as mybir
    import concourse.tile as tile
    from concourse import bacc
    from concourse.bass import ts

    f32 = mybir.dt.float32
    bf16 = mybir.dt.bfloat16
    AF = mybir.ActivationFunctionType
    OP = mybir.AluOpType
    P = 128

    nc = bacc.Bacc("TRN2", target_bir_lowering=False, debug=False,
                   enable_asserts=False, num_devices=8)

    def din(name, shape, dt):
        return nc.dram_tensor(name, shape, dt, kind="ExternalInput").ap()

    xsT = din("xsT", [2 * P, N], bf16)        # x_scalar^T, all tokens
    xqT = din("xqT", [2 * P, NQ], bf16)       # x_scalar^T, query half
    vq32 = din("vq32", [6 * P, NQ], f32)      # vec^T (query half), f32
    vq16 = din("vq16", [6 * P, NQ], bf16)     # vec^T (query half), bf16
    vkv = din("vkv", [N, 3 * HID], bf16)      # vec token-major, all tokens
    wqT = din("wqT", [HID, HID], bf16)
    wkT = din("wkT", [HID, HID], bf16)
    wvT = din("wvT", [HID, HID], bf16)
    wvecT = din("wvecT", [HID, 2 * HID], bf16)
    woT = din("woT", [HID, 3 * HID], bf16)
    wgT = din("wgT", [2 * HID, HID], bf16)    # alpha-folded
    bq = din("bq", [HID, 1], f32)
    bk = din("bk", [HID, 1], f32)
    bvB = din("bvB", [P, HID], f32)           # bv broadcast over partitions
    bg = din("bg", [HID, 1], f32)
    bo = din("bo", [3 * HID, 1], f32)
    ones = din("ones", [P, P], bf16)
    out = nc.dram_tensor("out", [4 * HID, NQ], f32, kind="ExternalOutput").ap()

    with tile.TileContext(nc) as tc:
        from contextlib import ExitStack
        with ExitStack() as ctx:
            def sb(name, shape, dt):
                return nc.alloc_sbuf_tensor("sb_" + name, list(shape), dt).ap()

            # ---------------- persistent SBUF ----------------
            xsT_s = [sb(f"xsT{i}", [P, N], bf16) for i in range(2)]
            xqT_s = [sb(f"xqT{i}", [P, NQ], bf16) for i in range(2)]
            vq16_s = [sb(f"vq16_{i}", [P, NQ], bf16) for i in range(6)]
            vq32_s = [sb(f"vq32_{i}", [P, NQ], f32) for i in range(6)]
            wq_s = [sb(f"wq{i}", [P, HID], bf16) for i in range(2)]
            wk_s = [sb(f"wk{i}", [P, HID], bf16) for i in range(2)]
            wv_s = [sb(f"wv{i}", [P, HID], bf16) for i in range(2)]
            wvec_s = [sb(f"wvec{i}", [P, 2 * HID], bf16) for i in range(2)]
            wo_s = [sb(f"wo{i}", [P, 3 * HID], bf16) for i in range(2)]
            wg_s = [sb(f"wg{i}", [P, HID], bf16) for i in range(4)]
            bq_s = [sb(f"bq{i}", [P, 1], f32) for i in range(2)]
            bk_s = [sb(f"bk{i}", [P, 1], f32) for i in range(2)]
            bg_s = [sb(f"bg{i}", [P, 1], f32) for i in range(2)]
            bo_s = [sb(f"bo{i}", [P, 1], f32) for i in range(6)]
            bvB_s = sb("bvB", [P, HID], f32)
            ones_s = sb("ones", [P, P], bf16)
            kT_s = [sb(f"kT{i}", [P, N], bf16) for i in range(2)]
            qT_s = [sb(f"qT{i}", [P, NQ], bf16) for i in range(2)]
            vall_s = [sb(f"vall{t}", [P, NH * P], bf16) for t in range(NKT)]
            dot_s = [sb(f"dot{j}", [P, NQ], bf16) for j in range(2)]
            norm_s = [sb(f"norm{j}", [P, NQ], bf16) for j in range(2)]
            gate_s = [sb(f"gate{j}", [P, NQ], f32) for j in range(2)]
            xout_s = [sb(f"xout{j}", [P, NQ], bf16) for j in range(2)]
            vaG_s = [[sb(f"vaG{c}_{j}", [P, NQ], f32) for j in range(2)]
                     for c in range(3)]

            dma = nc.sync.dma_start

            # ---------------- input DMAs ----------------
            for i in range(2):
                dma(out=xsT_s[i], in_=xsT[i * P:(i + 1) * P, :])
                dma(out=xqT_s[i], in_=xqT[i * P:(i + 1) * P, :])
                dma(out=wq_s[i], in_=wqT[i * P:(i + 1) * P, :])
                dma(out=wk_s[i], in_=wkT[i * P:(i + 1) * P, :])
                dma(out=wv_s[i], in_=wvT[i * P:(i + 1) * P, :])
                dma(out=wvec_s[i], in_=wvecT[i * P:(i + 1) * P, :])
                dma(out=wo_s[i], in_=woT[i * P:(i + 1) * P, :])
                dma(out=bq_s[i], in_=bq[i * P:(i + 1) * P, :])
                dma(out=bk_s[i], in_=bk[i * P:(i + 1) * P, :])
                dma(out=bg_s[i], in_=bg[i * P:(i + 1) * P, :])
            for i in range(4):
                dma(out=wg_s[i], in_=wgT[i * P:(i + 1) * P, :])
            for i in range(6):
                dma(out=bo_s[i], in_=bo[i * P:(i + 1) * P, :])
                dma(out=vq16_s[i], in_=vq16[i * P:(i + 1) * P, :])
                dma(out=vq32_s[i], in_=vq32[i * P:(i + 1) * P, :])
            dma(out=bvB_s, in_=bvB)
            dma(out=ones_s, in_=ones)

            # ---------------- Phase A: projections ----------------
            with tc.tile_pool(name="psA", bufs=3, space="PSUM") as psA, \
                 tc.tile_pool(name="vkvp", bufs=3) as vkvp, \
                 tc.tile_pool(name="vpp", bufs=2) as vpp, \
                 tc.tile_pool(name="tmpp", bufs=2) as tmpp:

                # k^T = Wk @ xs^T   (+bk), bf16, [256, 2048]
                for i in range(2):
                    for j in range(4):
                        ps = psA.tile([P, 512], f32, tag="psA", name="psk")
                        for ic in range(2):
                            nc.tensor.matmul(ps, wk_s[ic][:, ts(i, P)],
                                             xsT_s[ic][:, ts(j, 512)],
                                             start=(ic == 0), stop=(ic == 1))
                        nc.any.tensor_scalar(out=kT_s[i][:, ts(j, 512)], in0=ps,
                                             scalar1=bk_s[i], scalar2=None,
                                             op0=OP.add)
                # q^T = (Wq @ xq^T + bq) * SCALE, bf16, [256, 1024]
                for i in range(2):
                    for j in range(2):
                        ps = psA.tile([P, 512], f32, tag="psA", name="psq")
                        for ic in range(2):
                            nc.tensor.matmul(ps, wq_s[ic][:, ts(i, P)],
                                             xqT_s[ic][:, ts(j, 512)],
                                             start=(ic == 0), stop=(ic == 1))
                        nc.any.tensor_scalar(out=qT_s[i][:, ts(j, 512)], in0=ps,
                                             scalar1=bq_s[i], scalar2=SCALE,
                                             op0=OP.add, op1=OP.mult)

                # v token-major + v_all assembly
                for t in range(NKT):
                    vk = vkvp.tile([P, 3 * HID], bf16, tag="vk", name="vk")
                    dma(out=vk, in_=vkv[t * P:(t + 1) * P, :])
                    ps = psA.tile([P, HID], f32, tag="psV", name="psv")
                    for ic in range(2):
                        nc.tensor.matmul(ps, xsT_s[ic][:, ts(t, P)], wv_s[ic],
                                         start=(ic == 0), stop=(ic == 1))
                    va3 = vall_s[t].rearrange("p (h d) -> p h d", h=NH)
                    ps3 = ps.rearrange("p (h d) -> p h d", d=HD)
                    bv3 = bvB_s.rearrange("p (h d) -> p h d", d=HD)
                    nc.vector.tensor_tensor(out=va3[:, :, 0:HD], in0=ps3,
                                            in1=bv3, op=OP.add)
                    vk4 = vk.rearrange("p (c h d) -> p c h d", c=3, d=HD)
                    for c in range(3):
                        nc.vector.tensor_copy(
                            va3[:, :, HD + c * HD: 2 * HD + c * HD],
                            vk4[:, c])

                # vec_proj (query half) + vec_dot
                for c in range(3):
                    vp = []
                    for o in range(4):
                        vpt = vpp.tile([P, NQ], bf16, tag=f"vp{o}",
                                       name=f"vp{o}")
                        for n in range(2):
                            ps = psA.tile([P, 512], f32, tag="psA", name="psp")
                            for ic in range(2):
                                nc.tensor.matmul(
                                    ps, wvec_s[ic][:, ts(o, P)],
                                    vq16_s[2 * c + ic][:, ts(n, 512)],
                                    start=(ic == 0), stop=(ic == 1))
                            nc.any.tensor_copy(out=vpt[:, ts(n, 512)], in_=ps)
                        vp.append(vpt)
                    for j in range(2):
                        if c == 0:
                            nc.any.tensor_tensor(out=dot_s[j], in0=vp[j],
                                                 in1=vp[2 + j], op=OP.mult)
                        else:
                            m = tmpp.tile([P, NQ], bf16, tag="dtmp",
                                          name="dtmp")
                            nc.any.tensor_tensor(out=m, in0=vp[j],
                                                 in1=vp[2 + j], op=OP.mult)
                            nc.any.tensor_tensor(out=dot_s[j], in0=dot_s[j],
                                                 in1=m, op=OP.add)

                # vec_norm (query half)
                for j in range(2):
                    nt = tmpp.tile([P, NQ], bf16, tag="ntmp", name="ntmp")
                    nc.any.tensor_tensor(out=nt, in0=vq16_s[j], in1=vq16_s[j],
                                         op=OP.mult)
                    for c in (1, 2):
                        m = tmpp.tile([P, NQ], bf16, tag="ntmp2", name="ntmp2")
                        nc.any.tensor_tensor(out=m, in0=vq16_s[2 * c + j],
                                             in1=vq16_s[2 * c + j], op=OP.mult)
                        nc.any.tensor_tensor(out=nt, in0=nt, in1=m, op=OP.add)
                    nc.scalar.activation(norm_s[j], nt, AF.Sqrt)

                # gate = sigmoid(Wg_scaled @ [dot; norm] + bg)
                inv_tiles = [dot_s[0], dot_s[1], norm_s[0], norm_s[1]]
                for o in range(2):
                    for n in range(2):
                        ps = psA.tile([P, 512], f32, tag="psA", name="psg")
                        for ic in range(4):
                            nc.tensor.matmul(ps, wg_s[ic][:, ts(o, P)],
                                             inv_tiles[ic][:, ts(n, 512)],
                                             start=(ic == 0), stop=(ic == 3))
                        nc.scalar.activation(gate_s[o][:, ts(n, 512)], ps,
                                             AF.Sigmoid, bias=bg_s[o])

            # ---------------- Phase B: attention ----------------
            # Head-quads j=0 (heads 0-3) and j=1 (heads 4-7). Per (j, qc):
            #  - S^T matmuls row-packed in head pairs into psum_s [128,1024]
            #  - one exp per pair tile
            #  - PV + denominator column-packed (tile_position=(0,32m)) so
            #    head 4j+m lands on partitions 32m..32m+32 of shared psum
            #    accumulators: xo (out_s), va0-2 (vec aggr), dn (softmax den)
            with tc.tile_pool(name="psS", bufs=1, space="PSUM") as psS, \
                 tc.tile_pool(name="psAcc", bufs=1, space="PSUM") as psAcc, \
                 tc.tile_pool(name="expp", bufs=5) as expp, \
                 tc.tile_pool(name="rcpp", bufs=2) as rcpp, \
                 tc.tile_pool(name="vcp", bufs=3) as vcp:
                for j in range(2):
                    for qc in range(2):
                        xo = psAcc.tile([P, 512], f32, tag="xo", name="xo")
                        va = [psAcc.tile([P, 512], f32, tag=f"va{c}",
                                         name=f"va{c}") for c in range(3)]
                        dn = psAcc.tile([P, 512], f32, tag="dn", name="dn")

                        def emit_pv(kt, exs):
                            st = (kt == 0)
                            sp = (kt == NKT - 1)
                            quant = [(xo, 0)] + \
                                    [(va[c], HD + c * HD) for c in range(3)]
                            for tgt, off in quant:
                                for m in range(4):
                                    h = 4 * j + m
                                    nc.tensor.matmul(
                                        tgt[32 * m:32 * m + 32, :],
                                        vall_s[kt][:, h * P + off:
                                                   h * P + off + HD],
                                        exs[m // 2][:, ts(m % 2, 512)],
                                        start=st, stop=sp,
                                        tile_position=(0, 32 * m))
                            for m in range(4):
                                nc.tensor.matmul(
                                    dn[32 * m:32 * m + 32, :],
                                    ones_s[:, 0:HD],
                                    exs[m // 2][:, ts(m % 2, 512)],
                                    start=st, stop=sp,
                                    tile_position=(0, 32 * m))

                        pending = None
                        for kt in range(NKT):
                            cur = []
                            for pp in range(2):
                                ss = psS.tile([P, 1024], f32, tag="ss",
                                              name="ss")
                                for e in range(2):
                                    r = 2 * pp + e
                                    nc.tensor.matmul(
                                        ss[:, ts(e, 512)],
                                        kT_s[j][32 * r:32 * r + 32, ts(kt, P)],
                                        qT_s[j][32 * r:32 * r + 32,
                                                ts(qc, 512)],
                                        start=True, stop=True,
                                        tile_position=(32 * r, 0))
                                ex = expp.tile([P, 1024], bf16, tag="ex",
                                               name="ex")
                                nc.scalar.activation(ex, ss, AF.Exp)
                                cur.append(ex)
                                if pp == 0 and pending is not None:
                                    emit_pv(*pending)
                                    pending = None
                            pending = (kt, cur)
                        emit_pv(*pending)
                        rc = rcpp.tile([P, 512], f32, tag="rc", name="rc")
                        nc.vector.reciprocal_approx_fast(out=rc, in_=dn)
                        nc.vector.tensor_tensor(out=xout_s[j][:, ts(qc, 512)],
                                                in0=xo, in1=rc, op=OP.mult)
                        for c in range(3):
                            nc.vector.tensor_tensor(
                                out=vaG_s[c][j][:, ts(qc, 512)],
                                in0=va[c], in1=rc, op=OP.mult)
                    # gate * vec_aggr + vec for this head-quad (overlaps the
                    # next quad's attention on DVE/DMA)
                    for c in range(3):
                        for n in range(2):
                            t = vcp.tile([P, 512], f32, tag="vc", name="vc")
                            nc.vector.tensor_tensor(
                                out=t, in0=gate_s[j][:, ts(n, 512)],
                                in1=vaG_s[c][j][:, ts(n, 512)], op=OP.mult)
                            nc.vector.tensor_tensor(
                                out=t, in0=t,
                                in1=vq32_s[2 * c + j][:, ts(n, 512)],
                                op=OP.add)
                            r0_ = (1 + c) * HID + j * P
                            dma(out=out[r0_:r0_ + P, ts(n, 512)], in_=t)

            # ---------------- epilogue ----------------
            with tc.tile_pool(name="psE", bufs=2, space="PSUM") as psE, \
                 tc.tile_pool(name="outp", bufs=2) as outp:
                for j in range(2):
                    for n in range(2):
                        pso = [psE.tile([P, 512], f32, tag=f"po{k}",
                                        name=f"po{k}") for k in range(3)]
                        for k in range(3):
                            o_idx = 2 * k + j
                            for ic in range(2):
                                nc.tensor.matmul(pso[k],
                                                 wo_s[ic][:, ts(o_idx, P)],
                                                 xout_s[ic][:, ts(n, 512)],
                                                 start=(ic == 0),
                                                 stop=(ic == 1))
                        t1 = outp.tile([P, 512], f32, tag="t1", name="t1")
                        nc.vector.scalar_tensor_tensor(
                            out=t1, in0=pso[0], scalar=bo_s[j],
                            in1=dot_s[j][:, ts(n, 512)],
                            op0=OP.add, op1=OP.mult)
                        t2 = outp.tile([P, 512], f32, tag="t2", name="t2")
                        nc.vector.scalar_tensor_tensor(
                            out=t2, in0=pso[1], scalar=bo_s[2 + j],
                            in1=norm_s[j][:, ts(n, 512)],
                            op0=OP.add, op1=OP.mult)
                        nc.any.tensor_tensor(out=t1, in0=t1, in1=t2, op=OP.add)
                        xu = outp.tile([P, 512], f32, tag="xu", name="xu")
                        nc.vector.scalar_tensor_tensor(
                            out=xu, in0=pso[2], scalar=bo_s[4 + j], in1=t1,
                            op0=OP.add, op1=OP.add)
                        dma(out=out[j * P:(j + 1) * P, ts(n, 512)], in_=xu)


    nc.compile()
    return nc


def _get_nc():
    if "nc" not in _CACHE:
        _CACHE["nc"] = _build_nc()
    return _CACHE["nc"]


def _make_in_maps(inputs):
    x = np.asarray(inputs["x"], np.float32)
    Wq = np.asarray(inputs["Wq"], np.float32)
    Wk = np.asarray(inputs["Wk"], np.float32)
    Wv = np.asarray(inputs["Wv"], np.float32)
    Wvec = np.asarray(inputs["Wvec"], np.float32)
    Wo = np.asarray(inputs["Wo"], np.float32)
    Wg = np.asarray(inputs["Wg"], np.float32)
    bq = np.asarray(inputs["bq"], np.float32)
    bk = np.asarray(inputs["bk"], np.float32)
    bv = np.asarray(inputs["bv"], np.float32)
    bo = np.asarray(inputs["bo"], np.float32)
    bg = np.asarray(inputs["bg"], np.float32)
    a_d = float(np.asarray(inputs["alpha_dot"]))
    a_n = float(np.asarray(inputs["alpha_norm"]))

    wgT = Wg.T.copy()
    wgT[:HID, :] *= a_d
    wgT[HID:, :] *= a_n

    common = {
        "wqT": np.ascontiguousarray(Wq.T).astype(BF),
        "wkT": np.ascontiguousarray(Wk.T).astype(BF),
        "wvT": np.ascontiguousarray(Wv.T).astype(BF),
        "wvecT": np.ascontiguousarray(Wvec.T).astype(BF),
        "woT": np.ascontiguousarray(Wo.T).astype(BF),
        "wgT": np.ascontiguousarray(wgT).astype(BF),
        "bq": np.ascontiguousarray(bq.reshape(HID, 1)),
        "bk": np.ascontiguousarray(bk.reshape(HID, 1)),
        "bg": np.ascontiguousarray(bg.reshape(HID, 1)),
        "bo": np.ascontiguousarray(bo.reshape(3 * HID, 1)),
        "bvB": np.ascontiguousarray(np.broadcast_to(bv, (128, HID))),
        "ones": np.ones((128, 128), BF),
    }

    in_maps = []
    for core in range(8):
        b, qh = core // 2, core % 2
        qs = slice(qh * NQ, (qh + 1) * NQ)
        xsT = np.ascontiguousarray(x[b, :, 0, :].T)
        vq = x[b, qs, 1:, :].transpose(1, 2, 0).reshape(3 * HID, NQ)
        m = dict(common)
        m["xsT"] = xsT.astype(BF)
        m["xqT"] = np.ascontiguousarray(xsT[:, qs]).astype(BF)
        m["vq32"] = np.ascontiguousarray(vq)
        m["vq16"] = np.ascontiguousarray(vq).astype(BF)
        m["vkv"] = np.ascontiguousarray(
            x[b, :, 1:, :].reshape(N, 3 * HID)).astype(BF)
        in_maps.append(m)
    return in_maps


def _gather(results):
    x_final = np.empty((B, N, 4, HID), np.float32)
    for core, res in enumerate(results):
        b, qh = core // 2, core % 2
        qs = slice(qh * NQ, (qh + 1) * NQ)
        o = res["out"]                       # [1024 ch, 1024 q]
        for c in range(4):
            x_final[b, qs, c, :] = o[c * HID:(c + 1) * HID, :].T
    return x_final


def _run(inputs, trace=False):
    from concourse.bass_utils import run_bass_kernel_spmd
    nc = _get_nc()
    in_maps = _make_in_maps(inputs)
    res = run_bass_kernel_spmd(nc, in_maps, core_ids=list(range(8)),
                               trace=trace)
    return _gather(res.results), res


def kernel(**inputs):
    out, _ = _run(inputs, trace=False)
    return out


def _install_trace_hook():
    try:
        import antenv.axon_hooks as ah
    except ModuleNotFoundError:
        import importlib.util
        spec = importlib.util.spec_from_file_location(
            "antenv.axon_hooks", "/opt/trn_rl_repo/antenv/axon_hooks.py")
        ah = importlib.util.module_from_spec(spec)
        sys.modules["antenv.axon_hooks"] = ah
        spec.loader.exec_module(ah)
    if ah.get_axon_ntff_profile_hook() is None:
        from trn_agent_boot.trn_boot import _ntff_profile_via_ctypes
        ah.set_axon_ntff_profile_hook(
            _ntff_profile_via_ctypes("/opt/axon/libaxon_pjrt.so"))
    # avoid the cloud-bucket artifact upload in the trace path
    import concourse.bass_utils as bu
    bu.upload_artifacts = lambda tmpdir: tmpdir


def run_traced(inputs, tmpdir=None):
    _install_trace_hook()
    from concourse.bass_utils import run_bass_kernel_spmd
    nc = _get_nc()
    in_maps = _make_in_maps(inputs)
    res = run_bass_kernel_spmd(nc, in_maps, core_ids=list(range(8)),
                               trace=True, tmpdir=tmpdir)
    return _gather(res.results), res


# revision 10
# speedup vs baseline: 1.6422x; 1.0444x over previous
"""Trainium2 Bass kernel for EquivariantSelfAttention (B=4, N=2048, HID=256, 8 heads).

Sharding: 8 cores = 4 batches x 2 query-halves. Each core computes full
attention for one batch over its 1024 queries (all 2048 keys), plus the
per-token epilogue, fully locally (no collectives).

Device layout is channel-major ("transposed"): all transposes are done on the
host (numpy) during shard prep / output gather, so the device only runs
matmuls + elementwise work on [channels, tokens] tiles.
"""

import sys

if "/opt/trn_rl_repo" not in sys.path:
    sys.path.insert(0, "/opt/trn_rl_repo")

import numpy as np
import ml_dtypes

B, N, HID, NH, HD = 4, 2048, 256, 8, 32
NQ = N // 2          # queries per core
NKT = N // 128       # key tiles
SCALE = float(1.0 / np.sqrt(HD))
BF = ml_dtypes.bfloat16

_CACHE = {}


def _build_nc():
    import concourse.bass as bass
    import concourse.mybir as mybir
    import concourse.tile as tile
    from concourse import bacc
    from concourse.bass import ts

    f32 = mybir.dt.float32
    bf16 = mybir.dt.bfloat16
    AF = mybir.ActivationFunctionType
    OP = mybir.AluOpType
    P = 128

    nc = bacc.Bacc("TRN2", target_bir_lowering=False, debug=False,
                   enable_asserts=False, num_devices=8)

    def din(name, shape, dt):
        return nc.dram_tensor(name, shape, dt, kind="ExternalInput").ap()

    xsT = din("xsT", [2 * P, N], bf16)        # x_scalar^T, all tokens
    xqT = din("xqT", [2 * P, NQ], bf16)       # x_scalar^T, query half
    vq32 = din("vq32", [6 * P, NQ], f32)      # vec^T (query half), f32
    vq16 = din("vq16", [6 * P, NQ], bf16)     # vec^T (query half), bf16
    vkv = din("vkv", [N, 3 * HID], bf16)      # vec token-major, all tokens
    wqT = din("wqT", [HID, HID], bf16)
    wkT = din("wkT", [HID, HID], bf16)
    wvT = din("wvT", [HID, HID], bf16)
    wvecT = din("wvecT", [HID, 2 * HID], bf16)
    woT = din("woT", [HID, 3 * HID], bf16)
    wgT = din("wgT", [2 * HID, HID], bf16)    # alpha-folded
    bq = din("bq", [HID, 1], f32)
    bk = din("bk", [HID, 1], f32)
    bvB = din("bvB", [P, HID], f32)           # bv broadcast over partitions
    bg = din("bg", [HID, 1], f32)
    bo = din("bo", [3 * HID, 1], f32)
    ones = din("ones", [P, P], bf16)
    out = nc.dram_tensor("out", [4 * HID, NQ], f32, kind="ExternalOutput").ap()

    with tile.TileContext(nc) as tc:
        from contextlib import ExitStack
        with ExitStack() as ctx:
            def sb(name, shape, dt):
                return nc.alloc_sbuf_tensor("sb_" + name, list(shape), dt).ap()

            # ---------------- persistent SBUF ----------------
            xsT_s = [sb(f"xsT{i}", [P, N], bf16) for i in range(2)]
            xqT_s = [sb(f"xqT{i}", [P, NQ], bf16) for i in range(2)]
            vq16_s = [sb(f"vq16_{i}", [P, NQ], bf16) for i in range(6)]
            vq32_s = [sb(f"vq32_{i}", [P, NQ], f32) for i in range(6)]
            wq_s = [sb(f"wq{i}", [P, HID], bf16) for i in range(2)]
            wk_s = [sb(f"wk{i}", [P, HID], bf16) for i in range(2)]
            wv_s = [sb(f"wv{i}", [P, HID], bf16) for i in range(2)]
            wvec_s = [sb(f"wvec{i}", [P, 2 * HID], bf16) for i in range(2)]
            wo_s = [sb(f"wo{i}", [P, 3 * HID], bf16) for i in range(2)]
            wg_s = [sb(f"wg{i}", [P, HID], bf16) for i in range(4)]
            bq_s = [sb(f"bq{i}", [P, 1], f32) for i in range(2)]
            bk_s = [sb(f"bk{i}", [P, 1], f32) for i in range(2)]
            bg_s = [sb(f"bg{i}", [P, 1], f32) for i in range(2)]
            bo_s = [sb(f"bo{i}", [P, 1], f32) for i in range(6)]
            bvB_s = sb("bvB", [P, HID], f32)
            ones_s = sb("ones", [P, P], bf16)
            kT_s = [sb(f"kT{i}", [P, N], bf16) for i in range(2)]
            qT_s = [sb(f"qT{i}", [P, NQ], bf16) for i in range(2)]
            vall_s = [sb(f"vall{t}", [P, NH * P], bf16) for t in range(NKT)]
            dot_s = [sb(f"dot{j}", [P, NQ], bf16) for j in range(2)]
            norm_s = [sb(f"norm{j}", [P, NQ], bf16) for j in range(2)]
            gate_s = [sb(f"gate{j}", [P, NQ], f32) for j in range(2)]
            xout_s = [sb(f"xout{j}", [P, NQ], bf16) for j in range(2)]
            vaG_s = [[sb(f"vaG{c}_{j}", [P, NQ], f32) for j in range(2)]
                     for c in range(3)]

            dma = nc.sync.dma_start

            # ---------------- input DMAs ----------------
            # attention-critical tensors first: k/q projections + v_all
            # assembly gate the start of the attention stream.
            for i in range(2):
                dma(out=xsT_s[i], in_=xsT[i * P:(i + 1) * P, :])
                dma(out=xqT_s[i], in_=xqT[i * P:(i + 1) * P, :])
                dma(out=wk_s[i], in_=wkT[i * P:(i + 1) * P, :])
                dma(out=wq_s[i], in_=wqT[i * P:(i + 1) * P, :])
                dma(out=wv_s[i], in_=wvT[i * P:(i + 1) * P, :])
                dma(out=bq_s[i], in_=bq[i * P:(i + 1) * P, :])
                dma(out=bk_s[i], in_=bk[i * P:(i + 1) * P, :])
            dma(out=bvB_s, in_=bvB)
            dma(out=ones_s, in_=ones)
            for i in range(2):
                dma(out=wvec_s[i], in_=wvecT[i * P:(i + 1) * P, :])
            for i in range(6):
                dma(out=vq16_s[i], in_=vq16[i * P:(i + 1) * P, :])
            for i in range(4):
                dma(out=wg_s[i], in_=wgT[i * P:(i + 1) * P, :])
            for i in range(2):
                dma(out=wo_s[i], in_=woT[i * P:(i + 1) * P, :])
                dma(out=bg_s[i], in_=bg[i * P:(i + 1) * P, :])
            for i in range(6):
                dma(out=bo_s[i], in_=bo[i * P:(i + 1) * P, :])
                dma(out=vq32_s[i], in_=vq32[i * P:(i + 1) * P, :])

            # ---------------- Phase A: projections ----------------
            with tc.tile_pool(name="psA", bufs=3, space="PSUM") as psA, \
                 tc.tile_pool(name="vkvp", bufs=3) as vkvp, \
                 tc.tile_pool(name="vpp", bufs=2) as vpp, \
                 tc.tile_pool(name="tmpp", bufs=2) as tmpp:

                # k^T = Wk @ xs^T   (+bk), bf16, [256, 2048]
                for i in range(2):
                    for j in range(4):
                        ps = psA.tile([P, 512], f32, tag="psA", name="psk")
                        for ic in range(2):
                            nc.tensor.matmul(ps, wk_s[ic][:, ts(i, P)],
                                             xsT_s[ic][:, ts(j, 512)],
                                             start=(ic == 0), stop=(ic == 1))
                        nc.any.tensor_scalar(out=kT_s[i][:, ts(j, 512)], in0=ps,
                                             scalar1=bk_s[i], scalar2=None,
                                             op0=OP.add)
                # q^T = (Wq @ xq^T + bq) * SCALE, bf16, [256, 1024]
                for i in range(2):
                    for j in range(2):
                        ps = psA.tile([P, 512], f32, tag="psA", name="psq")
                        for ic in range(2):
                            nc.tensor.matmul(ps, wq_s[ic][:, ts(i, P)],
                                             xqT_s[ic][:, ts(j, 512)],
                                             start=(ic == 0), stop=(ic == 1))
                        nc.any.tensor_scalar(out=qT_s[i][:, ts(j, 512)], in0=ps,
                                             scalar1=bq_s[i], scalar2=SCALE,
                                             op0=OP.add, op1=OP.mult)

                # v token-major + v_all assembly
                for t in range(NKT):
                    vk = vkvp.tile([P, 3 * HID], bf16, tag="vk", name="vk")
                    dma(out=vk, in_=vkv[t * P:(t + 1) * P, :])
                    ps = psA.tile([P, HID], f32, tag="psV", name="psv")
                    for ic in range(2):
                        nc.tensor.matmul(ps, xsT_s[ic][:, ts(t, P)], wv_s[ic],
                                         start=(ic == 0), stop=(ic == 1))
                    va3 = vall_s[t].rearrange("p (h d) -> p h d", h=NH)
                    ps3 = ps.rearrange("p (h d) -> p h d", d=HD)
                    bv3 = bvB_s.rearrange("p (h d) -> p h d", d=HD)
                    nc.vector.tensor_tensor(out=va3[:, :, 0:HD], in0=ps3,
                                            in1=bv3, op=OP.add)
                    vk4 = vk.rearrange("p (c h d) -> p c h d", c=3, d=HD)
                    for c in range(3):
                        nc.vector.tensor_copy(
                            va3[:, :, HD + c * HD: 2 * HD + c * HD],
                            vk4[:, c])

                # vec_proj (query half) + vec_dot
                for c in range(3):
                    vp = []
                    for o in range(4):
                        vpt = vpp.tile([P, NQ], bf16, tag=f"vp{o}",
                                       name=f"vp{o}")
                        for n in range(2):
                            ps = psA.tile([P, 512], f32, tag="psA", name="psp")
                            for ic in range(2):
                                nc.tensor.matmul(
                                    ps, wvec_s[ic][:, ts(o, P)],
                                    vq16_s[2 * c + ic][:, ts(n, 512)],
                                    start=(ic == 0), stop=(ic == 1))
                            nc.any.tensor_copy(out=vpt[:, ts(n, 512)], in_=ps)
                        vp.append(vpt)
                    for j in range(2):
                        if c == 0:
                            nc.any.tensor_tensor(out=dot_s[j], in0=vp[j],
                                                 in1=vp[2 + j], op=OP.mult)
                        else:
                            m = tmpp.tile([P, NQ], bf16, tag="dtmp",
                                          name="dtmp")
                            nc.any.tensor_tensor(out=m, in0=vp[j],
                                                 in1=vp[2 + j], op=OP.mult)
                            nc.any.tensor_tensor(out=dot_s[j], in0=dot_s[j],
                                                 in1=m, op=OP.add)

                # vec_norm (query half)
                for j in range(2):
                    nt = tmpp.tile([P, NQ], bf16, tag="ntmp", name="ntmp")
                    nc.any.tensor_tensor(out=nt, in0=vq16_s[j], in1=vq16_s[j],
                                         op=OP.mult)
                    for c in (1, 2):
                        m = tmpp.tile([P, NQ], bf16, tag="ntmp2", name="ntmp2")
                        nc.any.tensor_tensor(out=m, in0=vq16_s[2 * c + j],
                                             in1=vq16_s[2 * c + j], op=OP.mult)
                        nc.any.tensor_tensor(out=nt, in0=nt, in1=m, op=OP.add)
                    nc.scalar.activation(norm_s[j], nt, AF.Sqrt)

                # gate = sigmoid(Wg_scaled @ [dot; norm] + bg)
                inv_tiles = [dot_s[0], dot_s[1], norm_s[0], norm_s[1]]
                for o in range(2):
                    for n in range(2):
                        ps = psA.tile([P, 512], f32, tag="psA", name="psg")
                        for ic in range(4):
                            nc.tensor.matmul(ps, wg_s[ic][:, ts(o, P)],
                                             inv_tiles[ic][:, ts(n, 512)],
                                             start=(ic == 0), stop=(ic == 3))
                        nc.scalar.activation(gate_s[o][:, ts(n, 512)], ps,
                                             AF.Sigmoid, bias=bg_s[o])

            # ---------------- Phase B: attention ----------------
            # Head-quads j=0 (heads 0-3) and j=1 (heads 4-7). Per (j, qc):
            #  - S^T matmuls row-packed in head pairs into psum_s [128,1024]
            #  - one exp per pair tile
            #  - PV + denominator column-packed (tile_position=(0,32m)) so
            #    head 4j+m lands on partitions 32m..32m+32 of shared psum
            #    accumulators: xo (out_s), va0-2 (vec aggr), dn (softmax den)
            with tc.tile_pool(name="psS", bufs=1, space="PSUM") as psS, \
                 tc.tile_pool(name="psAcc", bufs=1, space="PSUM") as psAcc, \
                 tc.tile_pool(name="expp", bufs=5) as expp, \
                 tc.tile_pool(name="rcpp", bufs=2) as rcpp, \
                 tc.tile_pool(name="vcp", bufs=3) as vcp:
                for j in range(2):
                    for qc in range(2):
                        xo = psAcc.tile([P, 512], f32, tag="xo", name="xo")
                        va = [psAcc.tile([P, 512], f32, tag=f"va{c}",
                                         name=f"va{c}") for c in range(3)]
                        dn = psAcc.tile([P, 512], f32, tag="dn", name="dn")

                        def emit_pv(kt, exs):
                            st = (kt == 0)
                            sp = (kt == NKT - 1)
                            quant = [(xo, 0)] + \
                                    [(va[c], HD + c * HD) for c in range(3)]
                            for tgt, off in quant:
                                for m in range(4):
                                    h = 4 * j + m
                                    nc.tensor.matmul(
                                        tgt[32 * m:32 * m + 32, :],
                                        vall_s[kt][:, h * P + off:
                                                   h * P + off + HD],
                                        exs[m // 2][:, ts(m % 2, 512)],
                                        start=st, stop=sp,
                                        tile_position=(0, 32 * m))
                            for m in range(4):
                                nc.tensor.matmul(
                                    dn[32 * m:32 * m + 32, :],
                                    ones_s[:, 0:HD],
                                    exs[m // 2][:, ts(m % 2, 512)],
                                    start=st, stop=sp,
                                    tile_position=(0, 32 * m))

                        pending = None
                        for kt in range(NKT):
                            cur = []
                            for pp in range(2):
                                ss = psS.tile([P, 1024], f32, tag="ss",
                                              name="ss")
                                for e in range(2):
                                    r = 2 * pp + e
                                    nc.tensor.matmul(
                                        ss[:, ts(e, 512)],
                                        kT_s[j][32 * r:32 * r + 32, ts(kt, P)],
                                        qT_s[j][32 * r:32 * r + 32,
                                                ts(qc, 512)],
                                        start=True, stop=True,
                                        tile_position=(32 * r, 0))
                                ex = expp.tile([P, 1024], bf16, tag="ex",
                                               name="ex")
                                nc.scalar.activation(ex, ss, AF.Exp)
                                cur.append(ex)
                                if pp == 0 and pending is not None:
                                    emit_pv(*pending)
                                    pending = None
                            pending = (kt, cur)
                        emit_pv(*pending)
                        rc = rcpp.tile([P, 512], f32, tag="rc", name="rc")
                        nc.vector.reciprocal_approx_fast(out=rc, in_=dn)
                        nc.vector.tensor_tensor(out=xout_s[j][:, ts(qc, 512)],
                                                in0=xo, in1=rc, op=OP.mult)
                        for c in range(3):
                            nc.vector.tensor_tensor(
                                out=vaG_s[c][j][:, ts(qc, 512)],
                                in0=va[c], in1=rc, op=OP.mult)
                    # gate * vec_aggr + vec for this head-quad (overlaps the
                    # next quad's attention on DVE/DMA)
                    for c in range(3):
                        for n in range(2):
                            t = vcp.tile([P, 512], f32, tag="vc", name="vc")
                            nc.vector.tensor_tensor(
                                out=t, in0=gate_s[j][:, ts(n, 512)],
                                in1=vaG_s[c][j][:, ts(n, 512)], op=OP.mult)
                            nc.vector.tensor_tensor(
                                out=t, in0=t,
                                in1=vq32_s[2 * c + j][:, ts(n, 512)],
                                op=OP.add)
                            r0_ = (1 + c) * HID + j * P
                            dma(out=out[r0_:r0_ + P, ts(n, 512)], in_=t)

            # ---------------- epilogue ----------------
            with tc.tile_pool(name="psE", bufs=2, space="PSUM") as psE, \
                 tc.tile_pool(name="outp", bufs=2) as outp:
                for j in range(2):
                    for n in range(2):
                        pso = [psE.tile([P, 512], f32, tag=f"po{k}",
                                        name=f"po{k}") for k in range(3)]
                        for k in range(3):
                            o_idx = 2 * k + j
                            for ic in range(2):
                                nc.tensor.matmul(pso[k],
                                                 wo_s[ic][:, ts(o_idx, P)],
                                                 xout_s[ic][:, ts(n, 512)],
                                                 start=(ic == 0),
                                                 stop=(ic == 1))
                        t1 = outp.tile([P, 512], f32, tag="t1", name="t1")
                        nc.vector.scalar_tensor_tensor(
                            out=t1, in0=pso[0], scalar=bo_s[j],
                            in1=dot_s[j][:, ts(n, 512)],
                            op0=OP.add, op1=OP.mult)
                        t2 = outp.tile([P, 512], f32, tag="t2", name="t2")
                        nc.vector.scalar_tensor_tensor(
                            out=t2, in0=pso[1], scalar=bo_s[2 + j],
                            in1=norm_s[j][:, ts(n, 512)],
                            op0=OP.add, op1=OP.mult)
                        nc.any.tensor_tensor(out=t1, in0=t1, in1=t2, op=OP.add)
                        xu = outp.tile([P, 512], f32, tag="xu", name="xu")
                        nc.vector.scalar_tensor_tensor(
                            out=xu, in0=pso[2], scalar=bo_s[4 + j], in1=t1,
                            op0=OP.add, op1=OP.add)
                        dma(out=out[j * P:(j + 1) * P, ts(n, 512)], in_=xu)


    nc.compile()
    return nc


def _get_nc():
    if "nc" not in _CACHE:
        _CACHE["nc"] = _build_nc()
    return _CACHE["nc"]


def _make_in_maps(inputs):
    x = np.asarray(inputs["x"], np.float32)
    Wq = np.asarray(inputs["Wq"], np.float32)
    Wk = np.asarray(inputs["Wk"], np.float32)
    Wv = np.asarray(inputs["Wv"], np.float32)
    Wvec = np.asarray(inputs["Wvec"], np.float32)
    Wo = np.asarray(inputs["Wo"], np.float32)
    Wg = np.asarray(inputs["Wg"], np.float32)
    bq = np.asarray(inputs["bq"], np.float32)
    bk = np.asarray(inputs["bk"], np.float32)
    bv = np.asarray(inputs["bv"], np.float32)
    bo = np.asarray(inputs["bo"], np.float32)
    bg = np.asarray(inputs["bg"], np.float32)
    a_d = float(np.asarray(inputs["alpha_dot"]))
    a_n = float(np.asarray(inputs["alpha_norm"]))

    wgT = Wg.T.copy()
    wgT[:HID, :] *= a_d
    wgT[HID:, :] *= a_n

    common = {
        "wqT": np.ascontiguousarray(Wq.T).astype(BF),
        "wkT": np.ascontiguousarray(Wk.T).astype(BF),
        "wvT": np.ascontiguousarray(Wv.T).astype(BF),
        "wvecT": np.ascontiguousarray(Wvec.T).astype(BF),
        "woT": np.ascontiguousarray(Wo.T).astype(BF),
        "wgT": np.ascontiguousarray(wgT).astype(BF),
        "bq": np.ascontiguousarray(bq.reshape(HID, 1)),
        "bk": np.ascontiguousarray(bk.reshape(HID, 1)),
        "bg": np.ascontiguousarray(bg.reshape(HID, 1)),
        "bo": np.ascontiguousarray(bo.reshape(3 * HID, 1)),
        "bvB": np.ascontiguousarray(np.broadcast_to(bv, (128, HID))),
        "ones": np.ones((128, 128), BF),
    }

    in_maps = []
    for core in range(8):
        b, qh = core // 2, core % 2
        qs = slice(qh * NQ, (qh + 1) * NQ)
        xsT = np.ascontiguousarray(x[b, :, 0, :].T)
        vq = x[b, qs, 1:, :].transpose(1, 2, 0).reshape(3 * HID, NQ)
        m = dict(common)
        m["xsT"] = xsT.astype(BF)
        m["xqT"] = np.ascontiguousarray(xsT[:, qs]).astype(BF)
        m["vq32"] = np.ascontiguousarray(vq)
        m["vq16"] = np.ascontiguousarray(vq).astype(BF)
        m["vkv"] = np.ascontiguousarray(
            x[b, :, 1:, :].reshape(N, 3 * HID)).astype(BF)
        in_maps.append(m)
    return in_maps


def _gather(results):
    x_final = np.empty((B, N, 4, HID), np.float32)
    for core, res in enumerate(results):
        b, qh = core // 2, core % 2
        qs = slice(qh * NQ, (qh + 1) * NQ)
        o = res["out"]                       # [1024 ch, 1024 q]
        for c in range(4):
            x_final[b, qs, c, :] = o[c * HID:(c + 1) * HID, :].T
    return x_final


def _run(inputs, trace=False):
    from concourse.bass_utils import run_bass_kernel_spmd
    nc = _get_nc()
    in_maps = _make_in_maps(inputs)
    res = run_bass_kernel_spmd(nc, in_maps, core_ids=list(range(8)),
                               trace=trace)
    return _gather(res.results), res


def kernel(**inputs):
    out, _ = _run(inputs, trace=False)
    return out


def _install_trace_hook():
    try:
        import antenv.axon_hooks as ah
    except ModuleNotFoundError:
        import importlib.util
        spec = importlib.util.spec_from_file_location(
            "antenv.axon_hooks", "/opt/trn_rl_repo/antenv/axon_hooks.py")
        ah = importlib.util.module_from_spec(spec)
        sys.modules["antenv.axon_hooks"] = ah
        spec.loader.exec_module(ah)
    if ah.get_axon_ntff_profile_hook() is None:
        from trn_agent_boot.trn_boot import _ntff_profile_via_ctypes
        ah.set_axon_ntff_profile_hook(
            _ntff_profile_via_ctypes("/opt/axon/libaxon_pjrt.so"))
    # avoid the cloud-bucket artifact upload in the trace path
    import concourse.bass_utils as bu
    bu.upload_artifacts = lambda tmpdir: tmpdir


def run_traced(inputs, tmpdir=None):
    _install_trace_hook()
    from concourse.bass_utils import run_bass_kernel_spmd
    nc = _get_nc()
    in_maps = _make_in_maps(inputs)
    res = run_bass_kernel_spmd(nc, in_maps, core_ids=list(range(8)),
                               trace=True, tmpdir=tmpdir)
    return _gather(res.results), res
